# revision 10
# baseline (speedup 1.0000x reference)
"""Pairwise-distance retrieval kernel (nn_Cov) for 8 Trainium2 NeuronCores.

Computes, for seq [N, D] with 0/1 masks qvs_idx/sum_idx:
    A = seq * qvs, B = seq * sum
    dist = sqrt(max(a2_i + b2_j - 2 A@B^T, eps))    [N, N]
    norm = dist.mean();  mn_i = min over valid j of dist_ij
    out = (1 - min(mn, norm)/norm) @ weight + bias  [N, 1]

Sharding: rows of A (queries) split across 8 cores; B replicated.
Valid (sum_idx=1) columns are permuted to the front on the host and padded
up to a 512 multiple with sentinel columns (bt = 0, b2 = B2PAD); columns
beyond that have B == 0 exactly, so their dist is sqrt(a2_i), added on the
host in closed form (the sentinel columns are likewise subtracted exactly).

Device per (128-row, up-to-2048-col 4-bank) psum tile:
  PE : psum = b2_j - 2*G_ij   via K=1 ones x b2 prefills (start=True)
       then 4 K=128 bf16 matmuls (A pre-scaled by -2), k-outer so
       consecutive matmuls share stationary weights (walrus ldw dedup).
  ACT: scr = fp16(Sqrt(psum + a2m_i)), accum_out -> per-row partial sums.
       No clamp is needed: a2m/b2 are computed on the host from the SAME
       bf16-rounded values the device multiplies (b2 rounded UP, a2 with
       a +0.5 margin), so psum + a2m >= 0 by construction.
  DVE: one tensor_tensor_reduce(min, min) folds the two halves of scr and
       reduces to a per-row partial min (in dist space).
The exact diagonal (the only near-duplicate pairs) is patched on the host,
which removes bf16 matmul noise from the min path.
"""

import os
import sys

import numpy as np

for _p in ("/opt/trn_rl_repo",):
    if os.path.isdir(_p) and _p not in sys.path:
        sys.path.insert(0, _p)

import concourse.bacc as bacc
import concourse.bass as bass
import concourse.bass_utils as _bass_utils
import concourse.mybir as mybir
import concourse.tile as tile
from concourse.bass_utils import run_bass_kernel_spmd

# Our k-outer matmul order issues runs of 4 matmuls sharing the same
# stationary weights; walrus's ldw dedup elides the redundant reloads.
if not getattr(_bass_utils, "_nn_cov_ldw_patch", False):
    _orig_gwa = _bass_utils.get_walrus_args

    def _gwa(*a, **k):
        return [
            x.replace("--enable-ldw-opt=false", "--enable-ldw-opt=true")
            if isinstance(x, str) else x
            for x in _orig_gwa(*a, **k)
        ]

    _bass_utils.get_walrus_args = _gwa
    _bass_utils._nn_cov_ldw_patch = True

N, D = 8192, 512
NCORES = 8
RPC = N // NCORES          # rows per core (1024)
MB = RPC // 128            # 128-row blocks per core (8)
CW = 512                   # column chunk width (one PSUM bank of fp32)
TW = 4 * CW                # psum tile width (4 banks)
KCH = D // 128             # contraction chunks (4)
EPS = 1e-12
B2PAD = 16384.0            # b2 sentinel for pad columns (dist ~128 >> real)
A2_MARGIN = 0.5            # keeps psum + a2m >= 0 under rounding

_BUILD_CACHE: dict = {}
LAST_RESULTS = None        # BassKernelResults of the most recent run


def _build(nvc: int):
    """Build + compile the SPMD Bass program for NVC 512-wide column chunks."""
    nc = bacc.Bacc("TRN2", target_bir_lowering=False)
    f32 = mybir.dt.float32
    f16 = mybir.dt.float16
    bf16 = mybir.dt.bfloat16
    AX = mybir.AxisListType.X
    OP = mybir.AluOpType
    Sqrt = mybir.ActivationFunctionType.Sqrt

    NPW = nvc * CW                     # processed (valid+pad) columns
    T = (nvc + 1) // 2                 # psum tiles per 128-row block
    tiles = [list(range(2 * t, min(2 * t + 2, nvc))) for t in range(T)]

    at_d = nc.dram_tensor("at0", [KCH, 128, RPC], bf16, kind="ExternalInput")
    bt_d = nc.dram_tensor("bt0", [KCH, 128, NPW], bf16, kind="ExternalInput")
    b2_d = nc.dram_tensor("b20", [1, NPW], bf16, kind="ExternalInput")
    a2_d = nc.dram_tensor("a20", [128, MB], f32, kind="ExternalInput")
    rmin_d = nc.dram_tensor("rmin0", [128, MB], f32, kind="ExternalOutput")
    rsum_d = nc.dram_tensor("rsum0", [128, MB], f32, kind="ExternalOutput")

    with tile.TileContext(nc) as tc:
        with (
            tc.tile_pool(name="big", bufs=1) as big,
            tc.tile_pool(name="work", bufs=4) as work,
            tc.tile_pool(name="psum", bufs=4, space="PSUM") as pp,
        ):
            # ACT table pre-warm: a dummy Sqrt during the DMA lead-in so the
            # 1.3us table load is off the critical path.
            dum_i = big.tile([128, 8], f32, name="dum_i", tag="dum_i")
            dum_o = big.tile([128, 8], f32, name="dum_o", tag="dum_o")
            nc.vector.memset(dum_i, 1.0)
            nc.scalar.activation(dum_o, dum_i, Sqrt)

            ones_sb = big.tile([1, 128], bf16, name="ones_sb", tag="ones")
            nc.vector.memset(ones_sb, 1.0)

            # DMA order: everything tile 0 needs first, k-interleaved.
            b2_sb = big.tile([1, NPW], bf16, name="b2_sb", tag="b2")
            nc.sync.dma_start(b2_sb, b2_d[:, :])
            a2_sb = big.tile([128, MB], f32, name="a2_sb", tag="a2")
            nc.sync.dma_start(a2_sb, a2_d[:, :])
            at_sb = [
                big.tile([128, RPC], bf16, name=f"at_sb{k}", tag=f"at{k}")
                for k in range(KCH)
            ]
            bt_sb = [
                big.tile([128, NPW], bf16, name=f"bt_sb{k}", tag=f"bt{k}")
                for k in range(KCH)
            ]
            t0hi = CW * len(tiles[0])
            for k in range(KCH):
                nc.sync.dma_start(at_sb[k], at_d[k])
                nc.sync.dma_start(bt_sb[k][:, :t0hi], bt_d[k][:, :t0hi])
            for t in range(1, T):
                lo = 2 * CW * t
                hi = lo + CW * len(tiles[t])
                for k in range(KCH):
                    nc.sync.dma_start(bt_sb[k][:, lo:hi], bt_d[k][:, lo:hi])

            sumbuf = big.tile([128, MB * T], f32, name="sumbuf", tag="sumbuf")
            minbuf = big.tile([128, MB * T], f32, name="minbuf", tag="minbuf")
            rmin_sb = big.tile([128, MB], f32, name="rmin_sb", tag="rmin")
            rsum_sb = big.tile([128, MB], f32, name="rsum_sb", tag="rsum")

            for t in range(T):
                chunks = tiles[t]
                w = CW * len(chunks)
                for m in range(MB):
                    ps = pp.tile([128, 2 * CW], f32, name="ps", tag="ps")
                    msl = slice(m * 128, (m + 1) * 128)
                    for i, c in enumerate(chunks):
                        nc.tensor.matmul(
                            ps[:, i * CW:(i + 1) * CW], ones_sb,
                            b2_sb[:, c * CW:(c + 1) * CW],
                            start=True, stop=False,
                        )
                    for k in range(KCH):
                        for i, c in enumerate(chunks):
                            nc.tensor.matmul(
                                ps[:, i * CW:(i + 1) * CW],
                                at_sb[k][:, msl],
                                bt_sb[k][:, c * CW:(c + 1) * CW],
                                start=False, stop=(k == KCH - 1),
                            )
                    idx = m * T + t
                    scr = work.tile([128, 2 * CW], f16, name="scr", tag="scr")
                    nc.scalar.activation(
                        scr[:, :w], ps[:, :w], Sqrt,
                        bias=a2_sb[:, m:m + 1],
                        accum_out=sumbuf[:, idx:idx + 1],
                    )
                    nc.vector.tensor_reduce(
                        minbuf[:, idx:idx + 1], scr[:, :w],
                        axis=AX, op=OP.min,
                    )
            nc.vector.tensor_reduce(
                rsum_sb, sumbuf.rearrange("p (m t) -> p m t", t=T),
                axis=AX, op=OP.add,
            )
            nc.vector.tensor_reduce(
                rmin_sb, minbuf.rearrange("p (m t) -> p m t", t=T),
                axis=AX, op=OP.min,
            )
            nc.sync.dma_start(rmin_d[:, :], rmin_sb)
            nc.sync.dma_start(rsum_d[:, :], rsum_sb)

    nc.compile()
    return nc


def kernel(seq, weight, bias, qvs_idx, sum_idx):
    global LAST_RESULTS
    import ml_dtypes

    bf16 = ml_dtypes.bfloat16

    seq = np.asarray(seq, dtype=np.float32)
    weight = np.asarray(weight, dtype=np.float32)
    bias = np.asarray(bias, dtype=np.float32)
    qvs_idx = np.asarray(qvs_idx, dtype=np.int32)
    sum_idx = np.asarray(sum_idx, dtype=np.int32)

    mq = (qvs_idx[:, 0] != 0)
    ms = (sum_idx[:, 0] != 0)
    A = seq * mq[:, None].astype(np.float32)
    B = seq * ms[:, None].astype(np.float32)
    a2 = np.einsum("nd,nd->n", A, A, dtype=np.float32).astype(np.float32)
    s2 = np.einsum("nd,nd->n", seq, seq, dtype=np.float32).astype(np.float32)

    # Stable permutation: valid (sum_idx=1) columns first.
    perm = np.argsort(~ms, kind="stable")
    NV = int(ms.sum())
    Bp = B[perm[:NV]] if NV else np.zeros((0, D), np.float32)

    NVC = max(1, -(-NV // CW))       # processed 512-col chunks (>=1)
    NPW = NVC * CW
    n_pad = NPW - NV

    # bf16-rounded operands; a2/b2 are computed from the SAME rounded
    # values the device multiplies, with b2 rounded UP in bf16, so
    # d2 = a2m + b2u - 2 A'.B' >= A2_MARGIN - fp32 drift > 0.
    at16 = (-2.0 * A).astype(bf16)                   # [N, D]
    Ap = at16.astype(np.float32) * np.float32(-0.5)  # exact A'
    a2p = np.einsum("nd,nd->n", Ap, Ap, dtype=np.float64).astype(np.float32)
    a2m = a2p + np.float32(A2_MARGIN)

    bt16 = Bp.astype(bf16)                           # [NV, D]
    Bpp = bt16.astype(np.float32)
    b2p = np.einsum("nd,nd->n", Bpp, Bpp, dtype=np.float64).astype(np.float32)
    b2u = b2p.astype(bf16)
    lowbits = b2u.view(np.uint16)
    lowbits[b2u.astype(np.float32) < b2p] += 1       # round UP (b2p > 0)
    b2f = np.zeros(NPW, dtype=bf16)
    b2f[:NV] = b2u
    b2f[NV:] = bf16(B2PAD)

    if NVC not in _BUILD_CACHE:
        _BUILD_CACHE[NVC] = _build(NVC)
    nc = _BUILD_CACHE[NVC]

    atT = np.ascontiguousarray(at16.T)               # [D, N]
    btp = np.zeros((D, NPW), dtype=bf16)
    if NV:
        btp[:, :NV] = bt16.T
    bt_chunks = btp.reshape(KCH, 128, NPW)

    in_maps = []
    for c in range(NCORES):
        csl = slice(c * RPC, (c + 1) * RPC)
        at_c = np.ascontiguousarray(atT[:, csl].reshape(KCH, 128, RPC))
        a2_c = np.ascontiguousarray(a2m[csl].reshape(MB, 128).T)
        in_maps.append({
            "at0": at_c,
            "bt0": bt_chunks,
            "b20": b2f.reshape(1, NPW),
            "a20": a2_c,
        })

    trace = bool(int(os.environ.get("NN_COV_TRACE", "0")))
    LAST_RESULTS = run_bass_kernel_spmd(
        nc, in_maps, core_ids=list(range(NCORES)), trace=trace
    )
    results = LAST_RESULTS.results

    row_min = np.empty(N, dtype=np.float32)   # min dist over processed cols
    row_sum = np.empty(N, dtype=np.float32)   # sum dist over processed cols
    for c in range(NCORES):
        row_min[c * RPC:(c + 1) * RPC] = results[c]["rmin0"].T.reshape(RPC)
        row_sum[c * RPC:(c + 1) * RPC] = results[c]["rsum0"].T.reshape(RPC)

    # Remove the sentinel pad columns (their device value is exact in
    # closed form) and add the (N - NV) invalid columns, where B == 0
    # exactly so dist_ij = sqrt(a2_i).
    if n_pad > 0:
        row_sum = row_sum - np.float32(n_pad) * np.sqrt(
            np.float32(B2PAD) + a2m
        ).astype(np.float32)
    if N - NV > 0:
        row_sum = row_sum + np.float32(N - NV) * np.sqrt(
            np.maximum(a2, np.float32(EPS))
        ).astype(np.float32)

    norm = np.float32(row_sum.sum(dtype=np.float64) / (float(N) * float(N)))

    # Patch the diagonal with its exact value: d2_ii = (mq XOR ms) * s2_i.
    # (The device's diag entry carries bf16 matmul rounding; the true value
    # is exact in closed form since A_i and B_i share seq_i.)
    d_diag = np.sqrt(np.maximum(
        np.where(mq ^ ms, s2, np.float32(0.0)), np.float32(EPS)
    )).astype(np.float32)
    if NV == 0:
        mn = np.full(N, np.inf, dtype=np.float32)
    else:
        mn = np.where(ms, np.minimum(row_min, d_diag), row_min)
    mn = np.minimum(mn, norm)
    simcov = (np.float32(1.0) - mn / norm).astype(np.float32)[:, None]
    out = simcov @ weight + bias[None, :]
    return out.astype(np.float32)


# revision 11
# speedup vs baseline: 1.0094x; 1.0094x over previous
"""Pairwise-distance retrieval kernel (nn_Cov) for 8 Trainium2 NeuronCores.

Computes, for seq [N, D] with 0/1 masks qvs_idx/sum_idx:
    A = seq * qvs, B = seq * sum
    dist = sqrt(max(a2_i + b2_j - 2 A@B^T, eps))    [N, N]
    norm = dist.mean();  mn_i = min over valid j of dist_ij
    out = (1 - min(mn, norm)/norm) @ weight + bias  [N, 1]

Sharding: rows of A (queries) split across 8 cores; B replicated.
Valid (sum_idx=1) columns are permuted to the front on the host and padded
up to a 512 multiple with sentinel columns (bt = 0, b2 = B2PAD); columns
beyond that have B == 0 exactly, so their dist is sqrt(a2_i), added on the
host in closed form (the sentinel columns are likewise subtracted exactly).

Device per (128-row, up-to-2048-col 4-bank) psum tile:
  PE : psum = b2_j - 2*G_ij   via K=1 ones x b2 prefills (start=True)
       then 4 K=128 bf16 matmuls (A pre-scaled by -2), k-outer so
       consecutive matmuls share stationary weights (walrus ldw dedup).
  ACT: scr = fp16(Sqrt(psum + a2m_i)), accum_out -> per-row partial sums.
       No clamp is needed: a2m/b2 are computed on the host from the SAME
       bf16-rounded values the device multiplies (b2 rounded UP, a2 with
       a +0.5 margin), so psum + a2m >= 0 by construction.
  DVE: one tensor_tensor_reduce(min, min) folds the two halves of scr and
       reduces to a per-row partial min (in dist space).
The exact diagonal (the only near-duplicate pairs) is patched on the host,
which removes bf16 matmul noise from the min path.
"""

import os
import sys

import numpy as np

for _p in ("/opt/trn_rl_repo",):
    if os.path.isdir(_p) and _p not in sys.path:
        sys.path.insert(0, _p)

import concourse.bacc as bacc
import concourse.bass as bass
import concourse.bass_utils as _bass_utils
import concourse.mybir as mybir
import concourse.tile as tile
from concourse.bass_utils import run_bass_kernel_spmd

# Our k-outer matmul order issues runs of 4 matmuls sharing the same
# stationary weights; walrus's ldw dedup elides the redundant reloads.
if not getattr(_bass_utils, "_nn_cov_ldw_patch", False):
    _orig_gwa = _bass_utils.get_walrus_args

    def _gwa(*a, **k):
        return [
            x.replace("--enable-ldw-opt=false", "--enable-ldw-opt=true")
            if isinstance(x, str) else x
            for x in _orig_gwa(*a, **k)
        ]

    _bass_utils.get_walrus_args = _gwa
    _bass_utils._nn_cov_ldw_patch = True

N, D = 8192, 512
NCORES = 8
RPC = N // NCORES          # rows per core (1024)
MB = RPC // 128            # 128-row blocks per core (8)
CW = 512                   # column chunk width (one PSUM bank of fp32)
TW = 4 * CW                # psum tile width (4 banks)
KCH = D // 128             # contraction chunks (4)
EPS = 1e-12
B2PAD = 16384.0            # b2 sentinel for pad columns (dist ~128 >> real)
A2_MARGIN = 0.5            # keeps psum + a2m >= 0 under rounding

_BUILD_CACHE: dict = {}
LAST_RESULTS = None        # BassKernelResults of the most recent run


def _build(nvc: int):
    """Build + compile the SPMD Bass program for NVC 512-wide column chunks."""
    nc = bacc.Bacc("TRN2", target_bir_lowering=False)
    f32 = mybir.dt.float32
    f16 = mybir.dt.float16
    bf16 = mybir.dt.bfloat16
    AX = mybir.AxisListType.X
    OP = mybir.AluOpType
    Sqrt = mybir.ActivationFunctionType.Sqrt

    NPW = nvc * CW                     # processed (valid+pad) columns
    T = (nvc + 1) // 2                 # psum tiles per 128-row block
    tiles = [list(range(2 * t, min(2 * t + 2, nvc))) for t in range(T)]

    at_d = nc.dram_tensor("at0", [KCH, 128, RPC], bf16, kind="ExternalInput")
    bt_d = nc.dram_tensor("bt0", [KCH, 128, NPW], bf16, kind="ExternalInput")
    b2_d = nc.dram_tensor("b20", [1, NPW], bf16, kind="ExternalInput")
    a2_d = nc.dram_tensor("a20", [128, MB], f32, kind="ExternalInput")
    rmin_d = nc.dram_tensor("rmin0", [128, MB], f32, kind="ExternalOutput")
    rsum_d = nc.dram_tensor("rsum0", [128, MB], f32, kind="ExternalOutput")

    with tile.TileContext(nc) as tc:
        with (
            tc.tile_pool(name="big", bufs=1) as big,
            tc.tile_pool(name="work", bufs=4) as work,
            tc.tile_pool(name="psum", bufs=4, space="PSUM") as pp,
        ):
            # ACT table pre-warm: a dummy Sqrt during the DMA lead-in so the
            # 1.3us table load is off the critical path.
            dum_i = big.tile([128, 8], f32, name="dum_i", tag="dum_i")
            dum_o = big.tile([128, 8], f32, name="dum_o", tag="dum_o")
            nc.vector.memset(dum_i, 1.0)
            nc.scalar.activation(dum_o, dum_i, Sqrt)

            ones_sb = big.tile([1, 128], bf16, name="ones_sb", tag="ones")
            nc.vector.memset(ones_sb, 1.0)

            # DMA order: everything tile 0 needs first, k-interleaved.
            b2_sb = big.tile([1, NPW], bf16, name="b2_sb", tag="b2")
            nc.sync.dma_start(b2_sb, b2_d[:, :])
            a2_sb = big.tile([128, MB], f32, name="a2_sb", tag="a2")
            nc.sync.dma_start(a2_sb, a2_d[:, :])
            at_sb = [
                big.tile([128, RPC], bf16, name=f"at_sb{k}", tag=f"at{k}")
                for k in range(KCH)
            ]
            bt_sb = [
                big.tile([128, NPW], bf16, name=f"bt_sb{k}", tag=f"bt{k}")
                for k in range(KCH)
            ]
            t0hi = CW * len(tiles[0])
            for k in range(KCH):
                nc.sync.dma_start(at_sb[k], at_d[k])
                nc.sync.dma_start(bt_sb[k][:, :t0hi], bt_d[k][:, :t0hi])
            for t in range(1, T):
                lo = 2 * CW * t
                hi = lo + CW * len(tiles[t])
                for k in range(KCH):
                    nc.sync.dma_start(bt_sb[k][:, lo:hi], bt_d[k][:, lo:hi])

            sumbuf = big.tile([128, MB * T], f32, name="sumbuf", tag="sumbuf")
            minbuf = big.tile([128, MB * T], f32, name="minbuf", tag="minbuf")
            rmin_sb = big.tile([128, MB], f32, name="rmin_sb", tag="rmin")
            rsum_sb = big.tile([128, MB], f32, name="rsum_sb", tag="rsum")

            for t in range(T):
                chunks = tiles[t]
                w = CW * len(chunks)
                for m in range(MB):
                    ps = pp.tile([128, 2 * CW], f32, name="ps", tag="ps")
                    msl = slice(m * 128, (m + 1) * 128)
                    for i, c in enumerate(chunks):
                        nc.tensor.matmul(
                            ps[:, i * CW:(i + 1) * CW], ones_sb,
                            b2_sb[:, c * CW:(c + 1) * CW],
                            start=True, stop=False,
                        )
                    for k in range(KCH):
                        for i, c in enumerate(chunks):
                            nc.tensor.matmul(
                                ps[:, i * CW:(i + 1) * CW],
                                at_sb[k][:, msl],
                                bt_sb[k][:, c * CW:(c + 1) * CW],
                                start=False, stop=(k == KCH - 1),
                            )
                    idx = m * T + t
                    scr = work.tile([128, 2 * CW], f16, name="scr", tag="scr")
                    # Alternate the row-sum between ACT's accumulator and a
                    # DVE reduce to keep the Scalar engine off the critical
                    # path (it otherwise lags PE into a long drain tail).
                    if idx % 2 == 0:
                        nc.scalar.activation(
                            scr[:, :w], ps[:, :w], Sqrt,
                            bias=a2_sb[:, m:m + 1],
                            accum_out=sumbuf[:, idx:idx + 1],
                        )
                    else:
                        nc.scalar.activation(
                            scr[:, :w], ps[:, :w], Sqrt,
                            bias=a2_sb[:, m:m + 1],
                        )
                        nc.vector.tensor_reduce(
                            sumbuf[:, idx:idx + 1], scr[:, :w],
                            axis=AX, op=OP.add,
                        )
                    nc.vector.tensor_reduce(
                        minbuf[:, idx:idx + 1], scr[:, :w],
                        axis=AX, op=OP.min,
                    )
            nc.vector.tensor_reduce(
                rsum_sb, sumbuf.rearrange("p (m t) -> p m t", t=T),
                axis=AX, op=OP.add,
            )
            nc.vector.tensor_reduce(
                rmin_sb, minbuf.rearrange("p (m t) -> p m t", t=T),
                axis=AX, op=OP.min,
            )
            nc.sync.dma_start(rmin_d[:, :], rmin_sb)
            nc.sync.dma_start(rsum_d[:, :], rsum_sb)

    nc.compile()
    return nc


def kernel(seq, weight, bias, qvs_idx, sum_idx):
    global LAST_RESULTS
    import ml_dtypes

    bf16 = ml_dtypes.bfloat16

    seq = np.asarray(seq, dtype=np.float32)
    weight = np.asarray(weight, dtype=np.float32)
    bias = np.asarray(bias, dtype=np.float32)
    qvs_idx = np.asarray(qvs_idx, dtype=np.int32)
    sum_idx = np.asarray(sum_idx, dtype=np.int32)

    mq = (qvs_idx[:, 0] != 0)
    ms = (sum_idx[:, 0] != 0)
    A = seq * mq[:, None].astype(np.float32)
    B = seq * ms[:, None].astype(np.float32)
    a2 = np.einsum("nd,nd->n", A, A, dtype=np.float32).astype(np.float32)
    s2 = np.einsum("nd,nd->n", seq, seq, dtype=np.float32).astype(np.float32)

    # Stable permutation: valid (sum_idx=1) columns first.
    perm = np.argsort(~ms, kind="stable")
    NV = int(ms.sum())
    Bp = B[perm[:NV]] if NV else np.zeros((0, D), np.float32)

    NVC = max(1, -(-NV // CW))       # processed 512-col chunks (>=1)
    NPW = NVC * CW
    n_pad = NPW - NV

    # bf16-rounded operands; a2/b2 are computed from the SAME rounded
    # values the device multiplies, with b2 rounded UP in bf16, so
    # d2 = a2m + b2u - 2 A'.B' >= A2_MARGIN - fp32 drift > 0.
    at16 = (-2.0 * A).astype(bf16)                   # [N, D]
    Ap = at16.astype(np.float32) * np.float32(-0.5)  # exact A'
    a2p = np.einsum("nd,nd->n", Ap, Ap, dtype=np.float64).astype(np.float32)
    a2m = a2p + np.float32(A2_MARGIN)

    bt16 = Bp.astype(bf16)                           # [NV, D]
    Bpp = bt16.astype(np.float32)
    b2p = np.einsum("nd,nd->n", Bpp, Bpp, dtype=np.float64).astype(np.float32)
    b2u = b2p.astype(bf16)
    lowbits = b2u.view(np.uint16)
    lowbits[b2u.astype(np.float32) < b2p] += 1       # round UP (b2p > 0)
    b2f = np.zeros(NPW, dtype=bf16)
    b2f[:NV] = b2u
    b2f[NV:] = bf16(B2PAD)

    if NVC not in _BUILD_CACHE:
        _BUILD_CACHE[NVC] = _build(NVC)
    nc = _BUILD_CACHE[NVC]

    atT = np.ascontiguousarray(at16.T)               # [D, N]
    btp = np.zeros((D, NPW), dtype=bf16)
    if NV:
        btp[:, :NV] = bt16.T
    bt_chunks = btp.reshape(KCH, 128, NPW)

    in_maps = []
    for c in range(NCORES):
        csl = slice(c * RPC, (c + 1) * RPC)
        at_c = np.ascontiguousarray(atT[:, csl].reshape(KCH, 128, RPC))
        a2_c = np.ascontiguousarray(a2m[csl].reshape(MB, 128).T)
        in_maps.append({
            "at0": at_c,
            "bt0": bt_chunks,
            "b20": b2f.reshape(1, NPW),
            "a20": a2_c,
        })

    trace = bool(int(os.environ.get("NN_COV_TRACE", "0")))
    LAST_RESULTS = run_bass_kernel_spmd(
        nc, in_maps, core_ids=list(range(NCORES)), trace=trace
    )
    results = LAST_RESULTS.results

    row_min = np.empty(N, dtype=np.float32)   # min dist over processed cols
    row_sum = np.empty(N, dtype=np.float32)   # sum dist over processed cols
    for c in range(NCORES):
        row_min[c * RPC:(c + 1) * RPC] = results[c]["rmin0"].T.reshape(RPC)
        row_sum[c * RPC:(c + 1) * RPC] = results[c]["rsum0"].T.reshape(RPC)

    # Remove the sentinel pad columns (their device value is exact in
    # closed form) and add the (N - NV) invalid columns, where B == 0
    # exactly so dist_ij = sqrt(a2_i).
    if n_pad > 0:
        row_sum = row_sum - np.float32(n_pad) * np.sqrt(
            np.float32(B2PAD) + a2m
        ).astype(np.float32)
    if N - NV > 0:
        row_sum = row_sum + np.float32(N - NV) * np.sqrt(
            np.maximum(a2, np.float32(EPS))
        ).astype(np.float32)

    norm = np.float32(row_sum.sum(dtype=np.float64) / (float(N) * float(N)))

    # Patch the diagonal with its exact value: d2_ii = (mq XOR ms) * s2_i.
    # (The device's diag entry carries bf16 matmul rounding; the true value
    # is exact in closed form since A_i and B_i share seq_i.)
    d_diag = np.sqrt(np.maximum(
        np.where(mq ^ ms, s2, np.float32(0.0)), np.float32(EPS)
    )).astype(np.float32)
    if NV == 0:
        mn = np.full(N, np.inf, dtype=np.float32)
    else:
        mn = np.where(ms, np.minimum(row_min, d_diag), row_min)
    mn = np.minimum(mn, norm)
    simcov = (np.float32(1.0) - mn / norm).astype(np.float32)[:, None]
    out = simcov @ weight + bias[None, :]
    return out.astype(np.float32)
